# revision 20
# baseline (speedup 1.0000x reference)
"""BertTinyFlatten on 8 Trainium2 NeuronCores — data-parallel over batch.

Per core (one batch element):
  emb   = gather(word_emb, x) + (pos_emb + tok_emb[0])      [indirect DMA w/ CCE add]
  x0    = layernorm(emb)                                     [token-major, DVE/ACT]
  x0t   = x0.T (PE transpose, LN affine fused into PSUM->SBUF copy, bf16)
  y1    = x0 @ init_d.T          (token-major out)           [bf16 matmuls]
  y1sq  = (mix(y1, init_M) + b1)^2    -> feature-major       [ACT Square fused]
  y2    = y1sq-chain @ inter0_d.T     -> token-major
  y2sq  = (mix(y2, inter0_M) + b2)^2  -> feature-major
  yt    = final_d-chain @ y2sq + b3   -> feature-major; host transposes back

All matmul operands are bf16 (PSUM accumulate in fp32): the moving
operand streams 1 col/cycle and the 128-col LDWEIGHTS rides the
background weight buffer, giving the 216 ns warm pitch at N=512.
Weights/M-panels are DMA'd as large host-prelaid panels (one trigger
per panel) because each dma_start costs ~0.64 us on the issuing
sequencer; triggers are split between the Sync and Scalar HWDGE
queues and ordered so the chunk-0 gather -> LN -> transpose -> MM
critical path and the w1 panel land first.
"""
import os
import sys

import numpy as np
import ml_dtypes

for _p in ("/opt/trn_rl_repo", "/opt/pypackages"):
    if _p not in sys.path and os.path.isdir(_p):
        sys.path.append(_p)

from contextlib import ExitStack

import concourse.bass as bass
import concourse.tile as tile
from concourse import bacc, masks, mybir
from concourse.bass import IndirectOffsetOnAxis
from concourse.bass_utils import run_bass_kernel_spmd

f32 = mybir.dt.float32
f32r = mybir.dt.float32r
bf16 = mybir.dt.bfloat16
i32 = mybir.dt.int32
AF = mybir.ActivationFunctionType
ALU = mybir.AluOpType
AX = mybir.AxisListType

B, S, HID, NH, INTER, VOCAB = 8, 1024, 512, 8, 2048, 30522
DH = INTER // NH            # 256 features per head
EPS = 1e-12
N_CORES = 8

KH = HID // 128             # 4   k-tiles for dense1
KI = INTER // 128           # 16  k-tiles for dense2/3
SC = S // 128               # 8   token chunks
NC1 = INTER // 512          # 4   n-chunks (512) for dense1/2
HT = HID // 128             # 4   hid tiles for dense3

STAGES = ("A", "B", "C", "D", "E", "full")


def _build_program(stage="full"):
    upto = STAGES.index(stage)
    nc = bacc.Bacc("TRN2", target_bir_lowering=False, debug=False,
                   num_devices=N_CORES, num_swdge_queues=2)

    xw = nc.dram_tensor("xw", [128, SC], i32, kind="ExternalInput").ap()
    word_emb = nc.dram_tensor("word_emb", [VOCAB, HID], f32, kind="ExternalInput").ap()
    posplus = nc.dram_tensor("posplus", [S, HID], f32, kind="ExternalInput").ap()
    # biases+ln catted: [0:16]=b1, [16:32]=b2, [32:36]=b3, [36:40]=lnw, [40:44]=lnb
    bcat = nc.dram_tensor("bcat", [128, 2 * KI + HT + 2 * HT], f32,
                          kind="ExternalInput").ap()
    # prelaid panels (host does the tiling): see _prep_maps for layouts
    w1 = nc.dram_tensor("w1", [NC1 * 128, KH * 512], bf16, kind="ExternalInput").ap()
    m1 = nc.dram_tensor("m1", [NH * 128, SC * S], bf16, kind="ExternalInput").ap()
    w2 = nc.dram_tensor("w2", [NC1 * 128, KI * 512], bf16, kind="ExternalInput").ap()
    m2 = nc.dram_tensor("m2", [NH * 128, SC * S], bf16, kind="ExternalInput").ap()
    w3 = nc.dram_tensor("w3", [128, KI * 512], bf16, kind="ExternalInput").ap()
    yt_out = nc.dram_tensor("yt", [HID, S], f32, kind="ExternalOutput").ap()

    with tile.TileContext(nc) as tc, ExitStack() as ctx:
        pool = ctx.enter_context(tc.tile_pool(name="sbuf", bufs=1))
        psum = ctx.enter_context(tc.tile_pool(name="psum", bufs=1, space="PSUM"))

        # ---- startup: critical path first -------------------------------
        t_idx = pool.tile([128, SC], i32)
        nc.sync.dma_start(t_idx[:], xw[:])

        emb = []
        for c in range(SC):
            e = pool.tile([128, HID], f32, tag="emb", bufs=SC, name=f"emb{c}")
            nc.sync.dma_start(e[:], posplus[c * 128:(c + 1) * 128, :])
            emb.append(e)

        # w1 panel, quarter n=0 first (dense1 s=0 n=0 needs it earliest)
        w1sb = pool.tile([128, NC1 * KH * 512], bf16)
        for n in range(NC1):
            nc.sync.dma_start(w1sb[:, n * 2048:(n + 1) * 2048],
                              w1[n * 128:(n + 1) * 128, :])

        # gathers (gpsimd swdge): chunk 0 split across both queues for latency
        def gather(out_ap, idx_ap, queue=None):
            gi = nc.gpsimd.indirect_dma_start(
                out=out_ap, out_offset=None,
                in_=word_emb[:out_ap.partition_size(), :],
                in_offset=IndirectOffsetOnAxis(ap=idx_ap, axis=0),
                bounds_check=VOCAB - 1, oob_is_err=False,
                compute_op=ALU.add,
            )
            if queue:
                gi.ins.queue = queue
            return gi

        for c in range(SC):
            gather(emb[c][:], t_idx[:, c:c + 1],
                   queue="qPoolDynamic1" if c % 2 else None)

        # ---- constants --------------------------------------------------
        ident = pool.tile([128, 128], f32)
        masks.make_identity(nc, ident[:])
        zerocol = pool.tile([128, 1], f32)
        nc.vector.memset(zerocol[:], 0.0)
        epscol = pool.tile([128, 1], f32)
        nc.vector.memset(epscol[:], EPS)
        t_bc = pool.tile([128, 2 * KI + 3 * HT], f32)
        nc.sync.dma_start(t_bc[:], bcat[:])
        # column bases into t_bc: b1, b2, b3, lnw, lnb
        B1, B2, B3, LNW, LNB = 0, KI, 2 * KI, 2 * KI + HT, 2 * KI + 2 * HT

        def bc(base, i):
            return t_bc[:, base + i:base + i + 1]

        # m1 heads 0/1 into the 2-slot panel ring
        def m_panel(m_ap, h, nm):
            t = pool.tile([128, SC * S], bf16, tag="m", bufs=3, name=f"{nm}p{h}")
            nc.sync.dma_start(t[:], m_ap[h * 128:(h + 1) * 128, :])
            return t

        m1pan = [m_panel(m1, 0, "m1"), m_panel(m1, 1, "m1"), m_panel(m1, 2, "m1")]

        # ---- stage A: per-chunk layernorm (invoked from fused dense1) --
        def layernorm_chunk(c):
            e = emb[c]
            msum = pool.tile([128, 1], f32, tag="msum", bufs=2, name=f"msum{c}")
            nc.vector.reduce_sum(msum[:], e[:], axis=AX.X)
            sqd = pool.tile([128, HID], f32, tag="sqd", bufs=2, name=f"sqd{c}")
            ssq = pool.tile([128, 1], f32, tag="ssq", bufs=2, name=f"ssq{c}")
            nc.scalar.activation(sqd[:], e[:], AF.Square, bias=zerocol[:],
                                 accum_out=ssq[:])
            # var = ssq/H - (msum/H)^2 ; rstd = rsqrt(var+eps)
            t1 = pool.tile([128, 1], f32, tag="t1", bufs=2, name=f"t1_{c}")
            nc.vector.tensor_scalar(t1[:], msum[:], msum[:], 1.0 / (HID * HID),
                                    op0=ALU.mult, op1=ALU.mult)
            var = pool.tile([128, 1], f32, tag="var", bufs=2, name=f"var{c}")
            nc.vector.tensor_scalar(var[:], ssq[:], 1.0 / HID, t1[:],
                                    op0=ALU.mult, op1=ALU.subtract)
            std = pool.tile([128, 1], f32, tag="std", bufs=2, name=f"std{c}")
            nc.scalar.activation(std[:], var[:], AF.Sqrt, bias=epscol[:])
            rstd = pool.tile([128, 1], f32, tag="rstd", bufs=2, name=f"rstd{c}")
            nc.vector.reciprocal(rstd[:], std[:])
            # e = e*rstd - (msum/H)*rstd
            mr = pool.tile([128, 1], f32, tag="mr", bufs=2, name=f"mr{c}")
            nc.vector.tensor_scalar(mr[:], msum[:], rstd[:], 1.0 / HID,
                                    op0=ALU.mult, op1=ALU.mult)
            nc.vector.tensor_scalar(e[:], e[:], rstd[:], mr[:],
                                    op0=ALU.mult, op1=ALU.subtract)

        # feature-major activations live in the 16-slot "featmaj" ring:
        # x0t (4 tiles) -> y1sq (16) -> y2sq (16), WAR-serialized by Tile.
        x0t = []
        for ht in range(HT):
            x0t.append(pool.tile([128, S], bf16, tag="featmaj", bufs=16,
                                 name=f"x0t{ht}"))

        def mix(yin, m_ap, panels, bias_base, out_name, tail_hooks=None):
            # per-head seq mix + bias + square; token-major in, feature-major out
            ysq = []
            for h in range(NH):
                pan = panels[h]
                groups = [[None] * 2 for _ in range(2)]
                for tc_i in range(2):
                    for dp in range(2):
                        groups[tc_i][dp] = psum.tile(
                            [128, 512], f32, tag="mm", bufs=8,
                            name=f"{out_name}p{h}_{tc_i}_{dp}")
                for s in range(SC):
                    for dp in range(2):
                        lhsT = yin[s][:, h * DH + dp * 128: h * DH + (dp + 1) * 128]
                        for tc_i in range(2):
                            nc.tensor.matmul(
                                groups[tc_i][dp][:], lhsT,
                                pan[:, s * S + tc_i * 512: s * S + (tc_i + 1) * 512],
                                start=(s == 0), stop=(s == SC - 1))
                if h + 3 < NH:
                    panels.append(m_panel(m_ap, h + 3, out_name))
                if tail_hooks and h in tail_hooks:
                    for fn in tail_hooks[h]:
                        fn()
                for dp in range(2):
                    i = h * 2 + dp
                    yo = pool.tile([128, S], bf16, tag="featmaj", bufs=16,
                                   name=f"{out_name}{i}")
                    for tc_i in range(2):
                        nc.scalar.activation(yo[:, tc_i * 512:(tc_i + 1) * 512],
                                             groups[tc_i][dp][:], AF.Square,
                                             bias=bc(bias_base, i))
                    ysq.append(yo)
            return ysq

        def w2_panel(n):
            t = pool.tile([128, KI * 512], bf16, tag="w2", bufs=2, name=f"w2q{n}")
            nc.sync.dma_start(t[:], w2[n * 128:(n + 1) * 128, :])
            return t

        def dense1(xin, transpose_src):
            # token-major out: y[s, n] = x @ w1; s-major with fused LN+transpose
            yt = []
            for s in range(SC):
                yt.append(pool.tile([128, INTER], bf16, tag="tokmaj", bufs=SC,
                                    name=f"y1_{s}"))
            for s in range(SC):
                layernorm_chunk(s)
                for ht in range(HT):
                    pt = psum.tile([128, 512], f32, tag="mm", bufs=8,
                                   name=f"ptr{ht}_{s}")
                    nc.tensor.transpose(
                        pt[:, 0:128],
                        transpose_src[s][:, ht * 128:(ht + 1) * 128],
                        ident[:])
                    nc.vector.tensor_scalar(
                        xin[ht][:, s * 128:(s + 1) * 128], pt[:, 0:128],
                        bc(LNW, ht), bc(LNB, ht),
                        op0=ALU.mult, op1=ALU.add)
                for n in range(NC1):
                    ps = psum.tile([128, 512], f32, tag="mm", bufs=8,
                                   name=f"y1p{n}_{s}")
                    for k in range(KH):
                        nc.tensor.matmul(
                            ps[:], xin[k][:, s * 128:(s + 1) * 128],
                            w1sb[:, n * 2048 + k * 512: n * 2048 + (k + 1) * 512],
                            start=(k == 0), stop=(k == KH - 1))
                    nc.scalar.copy(yt[s][:, n * 512:(n + 1) * 512], ps[:])
            return yt

        def dense2(xin, w2pans, tail_hooks=None):
            yt = []
            for s in range(SC):
                yt.append(pool.tile([128, INTER], bf16, tag="tokmaj", bufs=SC,
                                    name=f"y2_{s}"))
            for n in range(NC1):
                wq = w2pans[n]
                for s in range(SC):
                    ps = psum.tile([128, 512], f32, tag="mm", bufs=8,
                                   name=f"y2p{n}_{s}")
                    for k in range(KI):
                        nc.tensor.matmul(
                            ps[:], xin[k][:, s * 128:(s + 1) * 128],
                            wq[:, k * 512:(k + 1) * 512],
                            start=(k == 0), stop=(k == KI - 1))
                    nc.scalar.copy(yt[s][:, n * 512:(n + 1) * 512], ps[:])
                if n + 2 < NC1:
                    w2pans.append(w2_panel(n + 2))
                if tail_hooks and n in tail_hooks:
                    for fn in tail_hooks[n]:
                        fn()
            return yt

        def dump(tiles, width=S):
            # debug: write four [128, >=width] tiles to yt_out (via f32 staging)
            for i, t in enumerate(tiles[:4]):
                yo = pool.tile([128, S], f32, tag="out", bufs=2, name=f"dmp{i}")
                nc.vector.tensor_copy(yo[:, 0:width], t[:, 0:width])
                nc.sync.dma_start(yt_out[i * 128:(i + 1) * 128, 0:width],
                                  yo[:, 0:width])

        if upto == 0:                       # stage A only
            y1 = dense1(x0t, emb)
            dump(x0t)
        if upto >= 1:
            y1 = dense1(x0t, emb)
            if upto == 1:
                dump(y1)
        if upto >= 2:
            mix2_panels = []
            w2pans = []
            hooks1 = {
                5: [lambda: w2pans.append(w2_panel(0))],
                6: [lambda: w2pans.append(w2_panel(1))],
            }
            y1sq = mix(y1, m1, m1pan, B1, "y1sq", tail_hooks=hooks1)
            if upto == 2:
                dump(y1sq)
        if upto >= 3:
            w3sb = pool.tile([128, KI * 512], bf16)
            hooks2 = {
                0: [lambda: mix2_panels.append(m_panel(m2, 0, "m2"))],
                1: [lambda: mix2_panels.append(m_panel(m2, 1, "m2"))],
                2: [lambda: nc.sync.dma_start(w3sb[:], w3[:]),
                    lambda: mix2_panels.append(m_panel(m2, 2, "m2"))],
            }
            y2 = dense2(y1sq, w2pans, tail_hooks=hooks2)
            if upto == 3:
                dump(y2)
        if upto >= 4:
            y2sq = mix(y2, m2, mix2_panels, B2, "y2sq")
            if upto == 4:
                dump(y2sq)
        if upto >= 5:                       # dense3 + bias + store
            for ht in range(HT):
                yo = pool.tile([128, S], f32, tag="out", bufs=2, name=f"yt{ht}")
                pss = [psum.tile([128, 512], f32, tag="mm", bufs=8,
                                 name=f"p3_{ht}_{sc}") for sc in range(2)]
                for k in range(KI):
                    for sc in range(2):
                        nc.tensor.matmul(
                            pss[sc][:],
                            w3sb[:, k * 512 + ht * 128: k * 512 + ht * 128 + 128],
                            y2sq[k][:, sc * 512:(sc + 1) * 512],
                            start=(k == 0), stop=(k == KI - 1))
                for sc in range(2):
                    nc.scalar.activation(yo[:, sc * 512:(sc + 1) * 512], pss[sc][:],
                                         AF.Identity, bias=bc(B3, ht))
                    nc.sync.dma_start(
                        yt_out[ht * 128:(ht + 1) * 128, sc * 512:(sc + 1) * 512],
                        yo[:, sc * 512:(sc + 1) * 512])

    nc.compile()
    return nc


_PROGRAMS = {}
LAST_RESULT = None


def _get_program(stage="full"):
    if stage not in _PROGRAMS:
        _PROGRAMS[stage] = _build_program(stage)
    return _PROGRAMS[stage]


def _prep_maps(x, word_emb, pos_emb, tok_emb, emb_ln_w, emb_ln_b,
               init_d, init_b, init_M, inter0_d, inter0_b, inter0_M,
               final_d, final_b):
    x = np.asarray(x)
    f = lambda a: np.ascontiguousarray(np.asarray(a), dtype=np.float32)
    h = lambda a: np.ascontiguousarray(a).astype(ml_dtypes.bfloat16)
    w1h = f(init_d).reshape(NC1, 512, KH, 128).transpose(0, 3, 2, 1) \
        .reshape(NC1 * 128, KH * 512)
    w2h = f(inter0_d).reshape(NC1, 512, KI, 128).transpose(0, 3, 2, 1) \
        .reshape(NC1 * 128, KI * 512)
    w3h = f(final_d).reshape(HID, KI, 128).transpose(2, 1, 0) \
        .reshape(128, KI * 512)
    m1h = f(init_M).reshape(NH, SC, 128, S).transpose(0, 2, 1, 3) \
        .reshape(NH * 128, SC * S)
    m2h = f(inter0_M).reshape(NH, SC, 128, S).transpose(0, 2, 1, 3) \
        .reshape(NH * 128, SC * S)
    bcat = np.concatenate([
        f(init_b).reshape(KI, 128).T,
        f(inter0_b).reshape(KI, 128).T,
        f(final_b).reshape(HT, 128).T,
        f(emb_ln_w).reshape(HT, 128).T,
        f(emb_ln_b).reshape(HT, 128).T,
    ], axis=1)
    shared = dict(
        word_emb=f(word_emb),
        posplus=f(pos_emb) + f(tok_emb)[0][None, :],
        bcat=np.ascontiguousarray(bcat),
        w1=h(w1h), w2=h(w2h), w3=h(w3h), m1=h(m1h), m2=h(m2h),
    )
    in_maps = []
    for b in range(B):
        xwb = np.ascontiguousarray(x[b].astype(np.int32).reshape(SC, 128).T)
        in_maps.append(dict(shared, xw=xwb))
    return in_maps


def kernel(**inputs):
    global LAST_RESULT
    stage = os.environ.get("KSTAGE", "full")
    ncores = int(os.environ.get("KCORES", str(N_CORES)))
    in_maps = _prep_maps(**inputs)[:ncores]
    nc = _get_program(stage)
    res = run_bass_kernel_spmd(nc, in_maps, list(range(ncores)))
    LAST_RESULT = res
    out = np.stack([res.results[b]["yt"].T for b in range(ncores)])
    if ncores < B:
        out = np.concatenate([out] + [out[:1]] * (B - ncores))
    return out


# revision 31
# speedup vs baseline: 1.0311x; 1.0311x over previous
"""BertTinyFlatten on 8 Trainium2 NeuronCores — data-parallel over batch.

Per core (one batch element):
  emb   = gather(word_emb, x) + (pos_emb + tok_emb[0])      [indirect DMA w/ CCE add]
  x0    = layernorm(emb)                                     [token-major, DVE/ACT]
  x0t   = x0.T (PE transpose, LN affine fused into PSUM->SBUF copy, bf16)
  y1    = x0 @ init_d.T          (token-major out)           [bf16 matmuls]
  y1sq  = (mix(y1, init_M) + b1)^2    -> feature-major       [ACT Square fused]
  y2    = y1sq-chain @ inter0_d.T     -> token-major
  y2sq  = (mix(y2, inter0_M) + b2)^2  -> feature-major
  yt    = final_d-chain @ y2sq + b3   -> feature-major; host transposes back

All matmul operands are bf16 (PSUM accumulate in fp32): the moving
operand streams 1 col/cycle and the 128-col LDWEIGHTS rides the
background weight buffer, giving the 216 ns warm pitch at N=512.
Weights/M-panels are DMA'd as large host-prelaid panels (one trigger
per panel) because each dma_start costs ~0.64 us on the issuing
sequencer; triggers are split between the Sync and Scalar HWDGE
queues and ordered so the chunk-0 gather -> LN -> transpose -> MM
critical path and the w1 panel land first.
"""
import os
import sys

import numpy as np
import ml_dtypes

for _p in ("/opt/trn_rl_repo", "/opt/pypackages"):
    if _p not in sys.path and os.path.isdir(_p):
        sys.path.append(_p)

from contextlib import ExitStack

import concourse.bass as bass
import concourse.tile as tile
from concourse import bacc, masks, mybir
from concourse.bass import IndirectOffsetOnAxis
from concourse.bass_utils import run_bass_kernel_spmd

f32 = mybir.dt.float32
f32r = mybir.dt.float32r
bf16 = mybir.dt.bfloat16
i32 = mybir.dt.int32
AF = mybir.ActivationFunctionType
ALU = mybir.AluOpType
AX = mybir.AxisListType

B, S, HID, NH, INTER, VOCAB = 8, 1024, 512, 8, 2048, 30522
DH = INTER // NH            # 256 features per head
EPS = 1e-12
N_CORES = 8

KH = HID // 128             # 4   k-tiles for dense1
KI = INTER // 128           # 16  k-tiles for dense2/3
SC = S // 128               # 8   token chunks
NC1 = INTER // 512          # 4   n-chunks (512) for dense1/2
HT = HID // 128             # 4   hid tiles for dense3

STAGES = ("A", "B", "C", "D", "E", "full")


def _build_program(stage="full", affine=False):
    upto = STAGES.index(stage)
    nc = bacc.Bacc("TRN2", target_bir_lowering=False, debug=False,
                   num_devices=N_CORES, num_swdge_queues=2)

    xw = nc.dram_tensor("xw", [128, SC], i32, kind="ExternalInput").ap()
    word_emb = nc.dram_tensor("word_emb", [VOCAB, HID], f32, kind="ExternalInput").ap()
    posplus = nc.dram_tensor("posplus", [S, HID], f32, kind="ExternalInput").ap()
    # biases+ln catted: [0:16]=b1, [16:32]=b2, [32:36]=b3, [36:40]=lnw, [40:44]=lnb
    bcat = nc.dram_tensor("bcat", [128, 2 * KI + HT + 2 * HT], f32,
                          kind="ExternalInput").ap()
    # prelaid panels (host does the tiling): see _prep_maps for layouts
    w1 = nc.dram_tensor("w1", [NC1 * 128, KH * 512], bf16, kind="ExternalInput").ap()
    m1 = nc.dram_tensor("m1", [NH * 128, SC * S], bf16, kind="ExternalInput").ap()
    w2 = nc.dram_tensor("w2", [NC1 * 128, KI * 512], bf16, kind="ExternalInput").ap()
    m2 = nc.dram_tensor("m2", [NH * 128, SC * S], bf16, kind="ExternalInput").ap()
    w3 = nc.dram_tensor("w3", [128, KI * 512], bf16, kind="ExternalInput").ap()
    yt_out = nc.dram_tensor("yt", [HID, S], f32, kind="ExternalOutput").ap()

    with tile.TileContext(nc) as tc, ExitStack() as ctx:
        pool = ctx.enter_context(tc.tile_pool(name="sbuf", bufs=1))
        psum = ctx.enter_context(tc.tile_pool(name="psum", bufs=1, space="PSUM"))

        # ---- startup: critical path first -------------------------------
        t_idx = pool.tile([128, SC], i32)
        nc.sync.dma_start(t_idx[:], xw[:])

        # biases early: tiny payload, must not queue behind panel traffic
        t_bc = pool.tile([128, 2 * KI + 3 * HT], f32)
        nc.sync.dma_start(t_bc[:], bcat[:])

        emb = []
        for c in range(SC):
            e = pool.tile([128, HID], f32, tag="emb", bufs=SC, name=f"emb{c}")
            nc.sync.dma_start(e[:], posplus[c * 128:(c + 1) * 128, :])
            emb.append(e)

        # w1 panel, quarter n=0 first (dense1 s=0 n=0 needs it earliest)
        w1sb = pool.tile([128, NC1 * KH * 512], bf16)
        for n in range(NC1):
            nc.sync.dma_start(w1sb[:, n * 2048:(n + 1) * 2048],
                              w1[n * 128:(n + 1) * 128, :])

        # gathers (gpsimd swdge): chunk 0 split across both queues for latency
        def gather(out_ap, idx_ap, queue=None):
            gi = nc.gpsimd.indirect_dma_start(
                out=out_ap, out_offset=None,
                in_=word_emb[:out_ap.partition_size(), :],
                in_offset=IndirectOffsetOnAxis(ap=idx_ap, axis=0),
                bounds_check=VOCAB - 1, oob_is_err=False,
                compute_op=ALU.add,
            )
            if queue:
                gi.ins.queue = queue
            return gi

        # chunk 0: gather into its own tile (no CCE add) so it can run in
        # parallel with the posplus DMA; summed on DVE below.
        g0 = pool.tile([128, HID], f32)
        gather0 = nc.gpsimd.indirect_dma_start(
            out=g0[:], out_offset=None, in_=word_emb[:128, :],
            in_offset=IndirectOffsetOnAxis(ap=t_idx[:, 0:1], axis=0),
            bounds_check=VOCAB - 1, oob_is_err=False)
        for c in range(1, SC):
            gather(emb[c][:], t_idx[:, c:c + 1],
                   queue="qPoolDynamic1" if c % 2 else None)

        # ---- constants --------------------------------------------------
        ident = pool.tile([128, 128], f32)
        masks.make_identity(nc, ident[:])
        zerocol = pool.tile([128, 1], f32)
        nc.vector.memset(zerocol[:], 0.0)
        epscol = pool.tile([128, 1], f32)
        nc.vector.memset(epscol[:], EPS)
        # column bases into t_bc: b1, b2, b3, lnw, lnb
        B1, B2, B3, LNW, LNB = 0, KI, 2 * KI, 2 * KI + HT, 2 * KI + 2 * HT

        def bc(base, i):
            return t_bc[:, base + i:base + i + 1]

        # dummy Sqrt first so walrus picks the sqrt table set once (it also
        # contains Square/Identity) instead of reloading mid-layernorm
        actwarm = pool.tile([128, 1], f32)
        nc.scalar.activation(actwarm[:], epscol[:], AF.Sqrt, bias=zerocol[:])

        # PE warmup: ramp the clock during the otherwise idle startup window
        warmps = psum.tile([128, 512], f32, tag="mm", bufs=8, name="warm")
        for i in range(12):
            nc.tensor.transpose(warmps[:, 0:128], ident[:], ident[:])
        for i in range(8):
            nc.tensor.transpose(warmps[:, 0:128], g0[:, 0:128], ident[:])

        # m panels: DMA enqueue gated behind gather progress (via tiny gpsimd
        # copies into the slot) so 2MB panels don't delay gather payloads on
        # the shared DMA rings.
        def m_panel(m_ap, h, nm, gate=None):
            t = pool.tile([128, SC * S], bf16, tag="m", bufs=3, name=f"{nm}p{h}")
            if gate is not None:
                nc.gpsimd.tensor_copy(t[:, 0:1], gate[:, 0:1])
            nc.sync.dma_start(t[:], m_ap[h * 128:(h + 1) * 128, :])
            return t

        m1pan = [m_panel(m1, 0, "m1", gate=emb[3]),
                 m_panel(m1, 1, "m1", gate=emb[5]),
                 m_panel(m1, 2, "m1", gate=emb[7])]

        # ---- stage A: per-chunk layernorm (invoked from fused dense1) --
        def layernorm_chunk(c):
            e = emb[c]
            msum = pool.tile([128, 1], f32, tag="msum", bufs=2, name=f"msum{c}")
            if c == 0:
                # fused: e = e + g0 (gather part), msum = sum(e)
                nc.vector.scalar_tensor_tensor(e[:], e[:], 1.0, g0[:],
                                               op0=ALU.mult, op1=ALU.add,
                                               accum_out=msum[:])
            else:
                nc.vector.reduce_sum(msum[:], e[:], axis=AX.X)
            sqd = pool.tile([128, HID], f32, tag="sqd", bufs=2, name=f"sqd{c}")
            ssq = pool.tile([128, 1], f32, tag="ssq", bufs=2, name=f"ssq{c}")
            nc.scalar.activation(sqd[:], e[:], AF.Square, bias=zerocol[:],
                                 accum_out=ssq[:])
            # var = ssq/H - (msum/H)^2 ; rstd = rsqrt(var+eps)
            t1 = pool.tile([128, 1], f32, tag="t1", bufs=2, name=f"t1_{c}")
            nc.vector.tensor_scalar(t1[:], msum[:], msum[:], 1.0 / (HID * HID),
                                    op0=ALU.mult, op1=ALU.mult)
            var = pool.tile([128, 1], f32, tag="var", bufs=2, name=f"var{c}")
            nc.vector.tensor_scalar(var[:], ssq[:], 1.0 / HID, t1[:],
                                    op0=ALU.mult, op1=ALU.subtract)
            std = pool.tile([128, 1], f32, tag="std", bufs=2, name=f"std{c}")
            nc.scalar.activation(std[:], var[:], AF.Sqrt, bias=epscol[:])
            rstd = pool.tile([128, 1], f32, tag="rstd", bufs=2, name=f"rstd{c}")
            nc.vector.reciprocal(rstd[:], std[:])
            # e = e*rstd - (msum/H)*rstd
            mr = pool.tile([128, 1], f32, tag="mr", bufs=2, name=f"mr{c}")
            nc.vector.tensor_scalar(mr[:], msum[:], rstd[:], 1.0 / HID,
                                    op0=ALU.mult, op1=ALU.mult)
            nc.vector.tensor_scalar(e[:], e[:], rstd[:], mr[:],
                                    op0=ALU.mult, op1=ALU.subtract)

        # feature-major activations live in the 16-slot "featmaj" ring:
        # x0t (4 tiles) -> y1sq (16) -> y2sq (16), WAR-serialized by Tile.
        x0t = []
        for ht in range(HT):
            x0t.append(pool.tile([128, S], bf16, tag="featmaj", bufs=16,
                                 name=f"x0t{ht}"))

        def mix(yin, m_ap, panels, bias_base, out_name, tail_hooks=None):
            # per-head seq mix + bias + square; token-major in, feature-major out
            ysq = []
            for h in range(NH):
                pan = panels[h]
                groups = [[None] * 2 for _ in range(2)]
                for tc_i in range(2):
                    for dp in range(2):
                        groups[tc_i][dp] = psum.tile(
                            [128, 512], f32, tag="mm", bufs=8,
                            name=f"{out_name}p{h}_{tc_i}_{dp}")
                for s in range(SC):
                    for dp in range(2):
                        lhsT = yin[s][:, h * DH + dp * 128: h * DH + (dp + 1) * 128]
                        for tc_i in range(2):
                            nc.tensor.matmul(
                                groups[tc_i][dp][:], lhsT,
                                pan[:, s * S + tc_i * 512: s * S + (tc_i + 1) * 512],
                                start=(s == 0), stop=(s == SC - 1))
                if h + 3 < NH:
                    panels.append(m_panel(m_ap, h + 3, out_name))
                if tail_hooks and h in tail_hooks:
                    for fn in tail_hooks[h]:
                        fn()
                for dp in range(2):
                    i = h * 2 + dp
                    yo = pool.tile([128, S], bf16, tag="featmaj", bufs=16,
                                   name=f"{out_name}{i}")
                    for tc_i in range(2):
                        nc.scalar.activation(yo[:, tc_i * 512:(tc_i + 1) * 512],
                                             groups[tc_i][dp][:], AF.Square,
                                             bias=bc(bias_base, i))
                    ysq.append(yo)
            return ysq

        def w2_panel(n):
            t = pool.tile([128, KI * 512], bf16, tag="w2", bufs=2, name=f"w2q{n}")
            nc.sync.dma_start(t[:], w2[n * 128:(n + 1) * 128, :])
            return t

        def dense1(xin, transpose_src):
            # token-major out: y[s, n] = x @ w1; s-major with fused LN+transpose
            yt = []
            for s in range(SC):
                yt.append(pool.tile([128, INTER], bf16, tag="tokmaj", bufs=SC,
                                    name=f"y1_{s}"))
            for s in range(SC):
                layernorm_chunk(s)
                for ht in range(HT):
                    pt = psum.tile([128, 512], f32, tag="mm", bufs=8,
                                   name=f"ptr{ht}_{s}")
                    nc.tensor.transpose(
                        pt[:, 0:128],
                        transpose_src[s][:, ht * 128:(ht + 1) * 128],
                        ident[:])
                    if affine:
                        nc.vector.tensor_scalar(
                            xin[ht][:, s * 128:(s + 1) * 128], pt[:, 0:128],
                            bc(LNW, ht), bc(LNB, ht),
                            op0=ALU.mult, op1=ALU.add)
                    else:
                        # ln_w folded into w1 host-side; ln_b known zero
                        nc.vector.tensor_copy(
                            xin[ht][:, s * 128:(s + 1) * 128], pt[:, 0:128])
                for n in range(NC1):
                    ps = psum.tile([128, 512], f32, tag="mm", bufs=8,
                                   name=f"y1p{n}_{s}")
                    for k in range(KH):
                        nc.tensor.matmul(
                            ps[:], xin[k][:, s * 128:(s + 1) * 128],
                            w1sb[:, n * 2048 + k * 512: n * 2048 + (k + 1) * 512],
                            start=(k == 0), stop=(k == KH - 1))
                    nc.scalar.copy(yt[s][:, n * 512:(n + 1) * 512], ps[:])
            return yt

        def dense2(xin, w2pans, tail_hooks=None):
            yt = []
            for s in range(SC):
                yt.append(pool.tile([128, INTER], bf16, tag="tokmaj", bufs=SC,
                                    name=f"y2_{s}"))
            for n in range(NC1):
                wq = w2pans[n]
                for s in range(SC):
                    ps = psum.tile([128, 512], f32, tag="mm", bufs=8,
                                   name=f"y2p{n}_{s}")
                    for k in range(KI):
                        nc.tensor.matmul(
                            ps[:], xin[k][:, s * 128:(s + 1) * 128],
                            wq[:, k * 512:(k + 1) * 512],
                            start=(k == 0), stop=(k == KI - 1))
                    nc.scalar.copy(yt[s][:, n * 512:(n + 1) * 512], ps[:])
                if n + 2 < NC1:
                    w2pans.append(w2_panel(n + 2))
                if tail_hooks and n in tail_hooks:
                    for fn in tail_hooks[n]:
                        fn()
            return yt

        def dump(tiles, width=S):
            # debug: write four [128, >=width] tiles to yt_out (via f32 staging)
            for i, t in enumerate(tiles[:4]):
                yo = pool.tile([128, S], f32, tag="out", bufs=2, name=f"dmp{i}")
                nc.vector.tensor_copy(yo[:, 0:width], t[:, 0:width])
                nc.sync.dma_start(yt_out[i * 128:(i + 1) * 128, 0:width],
                                  yo[:, 0:width])

        if upto == 0:                       # stage A only
            y1 = dense1(x0t, emb)
            dump(x0t)
        if upto >= 1:
            y1 = dense1(x0t, emb)
            if upto == 1:
                dump(y1)
        if upto >= 2:
            mix2_panels = []
            w2pans = []
            hooks1 = {
                5: [lambda: w2pans.append(w2_panel(0))],
                6: [lambda: w2pans.append(w2_panel(1))],
            }
            y1sq = mix(y1, m1, m1pan, B1, "y1sq", tail_hooks=hooks1)
            if upto == 2:
                dump(y1sq)
        if upto >= 3:
            w3sb = pool.tile([128, KI * 512], bf16)
            hooks2 = {
                0: [lambda: mix2_panels.append(m_panel(m2, 0, "m2"))],
                1: [lambda: mix2_panels.append(m_panel(m2, 1, "m2"))],
                2: [lambda: nc.sync.dma_start(w3sb[:], w3[:]),
                    lambda: mix2_panels.append(m_panel(m2, 2, "m2"))],
            }
            y2 = dense2(y1sq, w2pans, tail_hooks=hooks2)
            if upto == 3:
                dump(y2)
        if upto >= 4:
            y2sq = mix(y2, m2, mix2_panels, B2, "y2sq")
            if upto == 4:
                dump(y2sq)
        if upto >= 5:                       # dense3 + bias + store
            for ht in range(HT):
                yo = pool.tile([128, S], f32, tag="out", bufs=2, name=f"yt{ht}")
                pss = [psum.tile([128, 512], f32, tag="mm", bufs=8,
                                 name=f"p3_{ht}_{sc}") for sc in range(2)]
                for k in range(KI):
                    for sc in range(2):
                        nc.tensor.matmul(
                            pss[sc][:],
                            w3sb[:, k * 512 + ht * 128: k * 512 + ht * 128 + 128],
                            y2sq[k][:, sc * 512:(sc + 1) * 512],
                            start=(k == 0), stop=(k == KI - 1))
                for sc in range(2):
                    nc.scalar.activation(yo[:, sc * 512:(sc + 1) * 512], pss[sc][:],
                                         AF.Identity, bias=bc(B3, ht))
                    nc.sync.dma_start(
                        yt_out[ht * 128:(ht + 1) * 128, sc * 512:(sc + 1) * 512],
                        yo[:, sc * 512:(sc + 1) * 512])

    nc.compile()
    return nc


_PROGRAMS = {}
LAST_RESULT = None


def _get_program(stage="full", affine=False):
    key = (stage, affine)
    if key not in _PROGRAMS:
        _PROGRAMS[key] = _build_program(stage, affine)
    return _PROGRAMS[key]


def _prep_maps(x, word_emb, pos_emb, tok_emb, emb_ln_w, emb_ln_b,
               init_d, init_b, init_M, inter0_d, inter0_b, inter0_M,
               final_d, final_b):
    x = np.asarray(x)
    f = lambda a: np.ascontiguousarray(np.asarray(a), dtype=np.float32)
    h = lambda a: np.ascontiguousarray(a).astype(ml_dtypes.bfloat16)
    affine = bool(np.any(np.asarray(emb_ln_b) != 0))
    w1f = f(init_d) if affine else f(init_d) * f(emb_ln_w)[None, :]
    w1h = w1f.reshape(NC1, 512, KH, 128).transpose(0, 3, 2, 1) \
        .reshape(NC1 * 128, KH * 512)
    w2h = f(inter0_d).reshape(NC1, 512, KI, 128).transpose(0, 3, 2, 1) \
        .reshape(NC1 * 128, KI * 512)
    w3h = f(final_d).reshape(HID, KI, 128).transpose(2, 1, 0) \
        .reshape(128, KI * 512)
    m1h = f(init_M).reshape(NH, SC, 128, S).transpose(0, 2, 1, 3) \
        .reshape(NH * 128, SC * S)
    m2h = f(inter0_M).reshape(NH, SC, 128, S).transpose(0, 2, 1, 3) \
        .reshape(NH * 128, SC * S)
    bcat = np.concatenate([
        f(init_b).reshape(KI, 128).T,
        f(inter0_b).reshape(KI, 128).T,
        f(final_b).reshape(HT, 128).T,
        f(emb_ln_w).reshape(HT, 128).T,
        f(emb_ln_b).reshape(HT, 128).T,
    ], axis=1)
    shared = dict(
        word_emb=f(word_emb),
        posplus=f(pos_emb) + f(tok_emb)[0][None, :],
        bcat=np.ascontiguousarray(bcat),
        w1=h(w1h), w2=h(w2h), w3=h(w3h), m1=h(m1h), m2=h(m2h),
    )
    in_maps = []
    for b in range(B):
        xwb = np.ascontiguousarray(x[b].astype(np.int32).reshape(SC, 128).T)
        in_maps.append(dict(shared, xw=xwb))
    return in_maps, affine


def kernel(**inputs):
    global LAST_RESULT
    stage = os.environ.get("KSTAGE", "full")
    ncores = int(os.environ.get("KCORES", str(N_CORES)))
    in_maps, affine = _prep_maps(**inputs)
    in_maps = in_maps[:ncores]
    nc = _get_program(stage, affine)
    res = run_bass_kernel_spmd(nc, in_maps, list(range(ncores)))
    LAST_RESULT = res
    out = np.stack([res.results[b]["yt"].T for b in range(ncores)])
    if ncores < B:
        out = np.concatenate([out] + [out[:1]] * (B - ncores))
    return out


# revision 34
# speedup vs baseline: 1.0737x; 1.0413x over previous
"""BertTinyFlatten on 8 Trainium2 NeuronCores — data-parallel over batch.

Per core (one batch element):
  emb   = gather(word_emb, x) + (pos_emb + tok_emb[0])      [indirect DMA w/ CCE add]
  x0    = layernorm(emb)                                     [token-major, DVE/ACT]
  x0t   = x0.T (PE transpose, LN affine fused into PSUM->SBUF copy, bf16)
  y1    = x0 @ init_d.T          (token-major out)           [bf16 matmuls]
  y1sq  = (mix(y1, init_M) + b1)^2    -> feature-major       [ACT Square fused]
  y2    = y1sq-chain @ inter0_d.T     -> token-major
  y2sq  = (mix(y2, inter0_M) + b2)^2  -> feature-major
  yt    = final_d-chain @ y2sq + b3   -> feature-major; host transposes back

All matmul operands are bf16 (PSUM accumulate in fp32): the moving
operand streams 1 col/cycle and the 128-col LDWEIGHTS rides the
background weight buffer, giving the 216 ns warm pitch at N=512.
Weights/M-panels are DMA'd as large host-prelaid panels (one trigger
per panel) because each dma_start costs ~0.64 us on the issuing
sequencer; triggers are split between the Sync and Scalar HWDGE
queues and ordered so the chunk-0 gather -> LN -> transpose -> MM
critical path and the w1 panel land first.
"""
import os
import sys

import numpy as np
import ml_dtypes

for _p in ("/opt/trn_rl_repo", "/opt/pypackages"):
    if _p not in sys.path and os.path.isdir(_p):
        sys.path.append(_p)

from contextlib import ExitStack

import concourse.bass as bass
import concourse.tile as tile
from concourse import bacc, masks, mybir
from concourse.bass import IndirectOffsetOnAxis
from concourse.bass_utils import run_bass_kernel_spmd

f32 = mybir.dt.float32
f32r = mybir.dt.float32r
bf16 = mybir.dt.bfloat16
i32 = mybir.dt.int32
AF = mybir.ActivationFunctionType
ALU = mybir.AluOpType
AX = mybir.AxisListType

B, S, HID, NH, INTER, VOCAB = 8, 1024, 512, 8, 2048, 30522
DH = INTER // NH            # 256 features per head
EPS = 1e-12
N_CORES = 8

KH = HID // 128             # 4   k-tiles for dense1
KI = INTER // 128           # 16  k-tiles for dense2/3
SC = S // 128               # 8   token chunks
NC1 = INTER // 512          # 4   n-chunks (512) for dense1/2
HT = HID // 128             # 4   hid tiles for dense3

STAGES = ("A", "B", "C", "D", "E", "full")


def _build_program(stage="full", affine=False):
    upto = STAGES.index(stage)
    nc = bacc.Bacc("TRN2", target_bir_lowering=False, debug=False,
                   num_devices=N_CORES, num_swdge_queues=2)

    xw = nc.dram_tensor("xw", [128, SC], i32, kind="ExternalInput").ap()
    word_emb = nc.dram_tensor("word_emb", [VOCAB, HID], f32, kind="ExternalInput").ap()
    posplus = nc.dram_tensor("posplus", [S, HID], f32, kind="ExternalInput").ap()
    # biases+ln catted: [0:16]=b1, [16:32]=b2, [32:36]=b3, [36:40]=lnw, [40:44]=lnb
    bcat = nc.dram_tensor("bcat", [128, 2 * KI + HT + 2 * HT], f32,
                          kind="ExternalInput").ap()
    # prelaid panels (host does the tiling): see _prep_maps for layouts
    w1 = nc.dram_tensor("w1", [NC1 * 128, KH * 512], bf16, kind="ExternalInput").ap()
    m1 = nc.dram_tensor("m1", [NH * 128, SC * S], bf16, kind="ExternalInput").ap()
    w2 = nc.dram_tensor("w2", [NC1 * 128, KI * 512], bf16, kind="ExternalInput").ap()
    m2 = nc.dram_tensor("m2", [NH * 128, SC * S], bf16, kind="ExternalInput").ap()
    w3 = nc.dram_tensor("w3", [128, KI * 512], bf16, kind="ExternalInput").ap()
    yt_out = nc.dram_tensor("yt", [HID, S], f32, kind="ExternalOutput").ap()

    with tile.TileContext(nc) as tc, ExitStack() as ctx:
        pool = ctx.enter_context(tc.tile_pool(name="sbuf", bufs=1))
        psum = ctx.enter_context(tc.tile_pool(name="psum", bufs=1, space="PSUM"))

        # ---- startup: critical path first -------------------------------
        t_idx = pool.tile([128, SC], i32)
        nc.sync.dma_start(t_idx[:], xw[:])

        # biases early: tiny payload, must not queue behind panel traffic
        t_bc = pool.tile([128, 2 * KI + 3 * HT], f32)
        nc.sync.dma_start(t_bc[:], bcat[:])

        emb = [pool.tile([128, HID], f32, tag="emb", bufs=SC, name=f"emb{c}")
               for c in range(SC)]
        nc.sync.dma_start(emb[0][:], posplus[0:128, :])

        # w1 panel next (quarter n=0 first); pp chunks 1-7 after it — their
        # gathers are serialized on the gpsimd descriptor builds anyway
        w1sb = pool.tile([128, NC1 * KH * 512], bf16)
        for n in range(NC1):
            nc.sync.dma_start(w1sb[:, n * 2048:(n + 1) * 2048],
                              w1[n * 128:(n + 1) * 128, :])
        for c in range(1, SC):
            nc.sync.dma_start(emb[c][:], posplus[c * 128:(c + 1) * 128, :])

        # gathers (gpsimd swdge): chunk 0 split across both queues for latency
        def gather(out_ap, idx_ap, queue=None):
            gi = nc.gpsimd.indirect_dma_start(
                out=out_ap, out_offset=None,
                in_=word_emb[:out_ap.partition_size(), :],
                in_offset=IndirectOffsetOnAxis(ap=idx_ap, axis=0),
                bounds_check=VOCAB - 1, oob_is_err=False,
                compute_op=ALU.add,
            )
            if queue:
                gi.ins.queue = queue
            return gi

        # chunk 0: gather into its own tile (no CCE add) so it can run in
        # parallel with the posplus DMA; summed on DVE below.
        g0 = pool.tile([128, HID], f32)
        gather0 = nc.gpsimd.indirect_dma_start(
            out=g0[:], out_offset=None, in_=word_emb[:128, :],
            in_offset=IndirectOffsetOnAxis(ap=t_idx[:, 0:1], axis=0),
            bounds_check=VOCAB - 1, oob_is_err=False)
        for c in range(1, SC):
            gather(emb[c][:], t_idx[:, c:c + 1],
                   queue="qPoolDynamic1" if c % 2 else None)

        # ---- constants --------------------------------------------------
        ident = pool.tile([128, 128], f32)
        masks.make_identity(nc, ident[:])
        zerocol = pool.tile([128, 1], f32)
        nc.vector.memset(zerocol[:], 0.0)
        epscol = pool.tile([128, 1], f32)
        nc.vector.memset(epscol[:], EPS)
        # column bases into t_bc: b1, b2, b3, lnw, lnb
        B1, B2, B3, LNW, LNB = 0, KI, 2 * KI, 2 * KI + HT, 2 * KI + 2 * HT

        def bc(base, i):
            return t_bc[:, base + i:base + i + 1]

        # dummy Sqrt first so walrus picks the sqrt table set once (it also
        # contains Square/Identity) instead of reloading mid-layernorm
        actwarm = pool.tile([128, 1], f32)
        nc.scalar.activation(actwarm[:], epscol[:], AF.Sqrt, bias=zerocol[:])

        # PE warmup: ramp the clock right before dense1 (g0-gated ones land
        # in the ~12-16us window, back-to-back with the first transposes)
        warmps = psum.tile([128, 512], f32, tag="mm", bufs=8, name="warm")
        for i in range(4):
            nc.tensor.transpose(warmps[:, 0:128], ident[:], ident[:])
        for i in range(16):
            nc.tensor.transpose(warmps[:, 0:128], g0[:, 0:128], ident[:])

        # m panels: DMA enqueue gated behind gather progress (via tiny gpsimd
        # copies into the slot) so 2MB panels don't delay gather payloads on
        # the shared DMA rings.
        def m_panel(m_ap, h, nm, gate=None):
            t = pool.tile([128, SC * S], bf16, tag="m", bufs=3, name=f"{nm}p{h}")
            if gate is not None:
                nc.gpsimd.tensor_copy(t[:, 0:1], gate[:, 0:1])
            nc.sync.dma_start(t[:], m_ap[h * 128:(h + 1) * 128, :])
            return t

        m1pan = [m_panel(m1, 0, "m1", gate=emb[7]),
                 m_panel(m1, 1, "m1", gate=emb[7]),
                 m_panel(m1, 2, "m1", gate=emb[7])]

        # ---- stage A: per-chunk layernorm (invoked from fused dense1) --
        def layernorm_chunk(c):
            e = emb[c]
            msum = pool.tile([128, 1], f32, tag="msum", bufs=2, name=f"msum{c}")
            if c == 0:
                # fused: e = e + g0 (gather part), msum = sum(e)
                nc.vector.scalar_tensor_tensor(e[:], e[:], 1.0, g0[:],
                                               op0=ALU.mult, op1=ALU.add,
                                               accum_out=msum[:])
            else:
                nc.vector.reduce_sum(msum[:], e[:], axis=AX.X)
            sqd = pool.tile([128, HID], f32, tag="sqd", bufs=2, name=f"sqd{c}")
            ssq = pool.tile([128, 1], f32, tag="ssq", bufs=2, name=f"ssq{c}")
            nc.scalar.activation(sqd[:], e[:], AF.Square, bias=zerocol[:],
                                 accum_out=ssq[:])
            # var = ssq/H - (msum/H)^2 ; rstd = rsqrt(var+eps)
            t1 = pool.tile([128, 1], f32, tag="t1", bufs=2, name=f"t1_{c}")
            nc.vector.tensor_scalar(t1[:], msum[:], msum[:], 1.0 / (HID * HID),
                                    op0=ALU.mult, op1=ALU.mult)
            var = pool.tile([128, 1], f32, tag="var", bufs=2, name=f"var{c}")
            nc.vector.tensor_scalar(var[:], ssq[:], 1.0 / HID, t1[:],
                                    op0=ALU.mult, op1=ALU.subtract)
            std = pool.tile([128, 1], f32, tag="std", bufs=2, name=f"std{c}")
            nc.scalar.activation(std[:], var[:], AF.Sqrt, bias=epscol[:])
            rstd = pool.tile([128, 1], f32, tag="rstd", bufs=2, name=f"rstd{c}")
            nc.vector.reciprocal(rstd[:], std[:])
            # e = e*rstd - (msum/H)*rstd
            mr = pool.tile([128, 1], f32, tag="mr", bufs=2, name=f"mr{c}")
            nc.vector.tensor_scalar(mr[:], msum[:], rstd[:], 1.0 / HID,
                                    op0=ALU.mult, op1=ALU.mult)
            nc.vector.tensor_scalar(e[:], e[:], rstd[:], mr[:],
                                    op0=ALU.mult, op1=ALU.subtract)

        # feature-major activations live in the 16-slot "featmaj" ring:
        # x0t (4 tiles) -> y1sq (16) -> y2sq (16), WAR-serialized by Tile.
        x0t = []
        for ht in range(HT):
            x0t.append(pool.tile([128, S], bf16, tag="featmaj", bufs=16,
                                 name=f"x0t{ht}"))

        def mix(yin, m_ap, panels, bias_base, out_name, tail_hooks=None):
            # per-head seq mix + bias + square; token-major in, feature-major out
            ysq = []
            for h in range(NH):
                pan = panels[h]
                groups = [[None] * 2 for _ in range(2)]
                for tc_i in range(2):
                    for dp in range(2):
                        groups[tc_i][dp] = psum.tile(
                            [128, 512], f32, tag="mm", bufs=8,
                            name=f"{out_name}p{h}_{tc_i}_{dp}")
                for s in range(SC):
                    for dp in range(2):
                        lhsT = yin[s][:, h * DH + dp * 128: h * DH + (dp + 1) * 128]
                        for tc_i in range(2):
                            nc.tensor.matmul(
                                groups[tc_i][dp][:], lhsT,
                                pan[:, s * S + tc_i * 512: s * S + (tc_i + 1) * 512],
                                start=(s == 0), stop=(s == SC - 1))
                if h + 3 < NH:
                    panels.append(m_panel(m_ap, h + 3, out_name))
                if tail_hooks and h in tail_hooks:
                    for fn in tail_hooks[h]:
                        fn()
                for dp in range(2):
                    i = h * 2 + dp
                    yo = pool.tile([128, S], bf16, tag="featmaj", bufs=16,
                                   name=f"{out_name}{i}")
                    for tc_i in range(2):
                        nc.scalar.activation(yo[:, tc_i * 512:(tc_i + 1) * 512],
                                             groups[tc_i][dp][:], AF.Square,
                                             bias=bc(bias_base, i))
                    ysq.append(yo)
            return ysq

        def w2_panel(n):
            t = pool.tile([128, KI * 512], bf16, tag="w2", bufs=2, name=f"w2q{n}")
            nc.sync.dma_start(t[:], w2[n * 128:(n + 1) * 128, :])
            return t

        def dense1(xin, transpose_src):
            # token-major out: y[s, n] = x @ w1; s-major with fused LN+transpose
            yt = []
            for s in range(SC):
                yt.append(pool.tile([128, INTER], bf16, tag="tokmaj", bufs=SC,
                                    name=f"y1_{s}"))
            for s in range(SC):
                layernorm_chunk(s)
                for ht in range(HT):
                    pt = psum.tile([128, 512], f32, tag="mm", bufs=8,
                                   name=f"ptr{ht}_{s}")
                    nc.tensor.transpose(
                        pt[:, 0:128],
                        transpose_src[s][:, ht * 128:(ht + 1) * 128],
                        ident[:])
                    if affine:
                        nc.vector.tensor_scalar(
                            xin[ht][:, s * 128:(s + 1) * 128], pt[:, 0:128],
                            bc(LNW, ht), bc(LNB, ht),
                            op0=ALU.mult, op1=ALU.add)
                    else:
                        # ln_w folded into w1 host-side; ln_b known zero
                        nc.vector.tensor_copy(
                            xin[ht][:, s * 128:(s + 1) * 128], pt[:, 0:128])
                for n in range(NC1):
                    ps = psum.tile([128, 512], f32, tag="mm", bufs=8,
                                   name=f"y1p{n}_{s}")
                    for k in range(KH):
                        nc.tensor.matmul(
                            ps[:], xin[k][:, s * 128:(s + 1) * 128],
                            w1sb[:, n * 2048 + k * 512: n * 2048 + (k + 1) * 512],
                            start=(k == 0), stop=(k == KH - 1))
                    nc.scalar.copy(yt[s][:, n * 512:(n + 1) * 512], ps[:])
            return yt

        def dense2(xin, w2pans, tail_hooks=None):
            yt = []
            for s in range(SC):
                yt.append(pool.tile([128, INTER], bf16, tag="tokmaj", bufs=SC,
                                    name=f"y2_{s}"))
            for n in range(NC1):
                wq = w2pans[n]
                for s in range(SC):
                    ps = psum.tile([128, 512], f32, tag="mm", bufs=8,
                                   name=f"y2p{n}_{s}")
                    for k in range(KI):
                        nc.tensor.matmul(
                            ps[:], xin[k][:, s * 128:(s + 1) * 128],
                            wq[:, k * 512:(k + 1) * 512],
                            start=(k == 0), stop=(k == KI - 1))
                    nc.scalar.copy(yt[s][:, n * 512:(n + 1) * 512], ps[:])
                if n + 2 < NC1:
                    w2pans.append(w2_panel(n + 2))
                if tail_hooks and n in tail_hooks:
                    for fn in tail_hooks[n]:
                        fn()
            return yt

        def dump(tiles, width=S):
            # debug: write four [128, >=width] tiles to yt_out (via f32 staging)
            for i, t in enumerate(tiles[:4]):
                yo = pool.tile([128, S], f32, tag="out", bufs=2, name=f"dmp{i}")
                nc.vector.tensor_copy(yo[:, 0:width], t[:, 0:width])
                nc.sync.dma_start(yt_out[i * 128:(i + 1) * 128, 0:width],
                                  yo[:, 0:width])

        if upto == 0:                       # stage A only
            y1 = dense1(x0t, emb)
            dump(x0t)
        if upto >= 1:
            y1 = dense1(x0t, emb)
            if upto == 1:
                dump(y1)
        if upto >= 2:
            mix2_panels = []
            w2pans = []
            hooks1 = {
                5: [lambda: w2pans.append(w2_panel(0))],
                6: [lambda: w2pans.append(w2_panel(1))],
            }
            y1sq = mix(y1, m1, m1pan, B1, "y1sq", tail_hooks=hooks1)
            if upto == 2:
                dump(y1sq)
        if upto >= 3:
            w3sb = pool.tile([128, KI * 512], bf16)
            hooks2 = {
                0: [lambda: mix2_panels.append(m_panel(m2, 0, "m2"))],
                1: [lambda: mix2_panels.append(m_panel(m2, 1, "m2"))],
                2: [lambda: nc.sync.dma_start(w3sb[:], w3[:]),
                    lambda: mix2_panels.append(m_panel(m2, 2, "m2"))],
            }
            y2 = dense2(y1sq, w2pans, tail_hooks=hooks2)
            if upto == 3:
                dump(y2)
        if upto >= 4:
            y2sq = mix(y2, m2, mix2_panels, B2, "y2sq")
            if upto == 4:
                dump(y2sq)
        if upto >= 5:                       # dense3 + bias + store
            for ht in range(HT):
                yo = pool.tile([128, S], f32, tag="out", bufs=2, name=f"yt{ht}")
                pss = [psum.tile([128, 512], f32, tag="mm", bufs=8,
                                 name=f"p3_{ht}_{sc}") for sc in range(2)]
                for k in range(KI):
                    for sc in range(2):
                        nc.tensor.matmul(
                            pss[sc][:],
                            w3sb[:, k * 512 + ht * 128: k * 512 + ht * 128 + 128],
                            y2sq[k][:, sc * 512:(sc + 1) * 512],
                            start=(k == 0), stop=(k == KI - 1))
                for sc in range(2):
                    nc.scalar.activation(yo[:, sc * 512:(sc + 1) * 512], pss[sc][:],
                                         AF.Identity, bias=bc(B3, ht))
                    nc.sync.dma_start(
                        yt_out[ht * 128:(ht + 1) * 128, sc * 512:(sc + 1) * 512],
                        yo[:, sc * 512:(sc + 1) * 512])

    nc.compile()
    return nc


_PROGRAMS = {}
LAST_RESULT = None


def _get_program(stage="full", affine=False):
    key = (stage, affine)
    if key not in _PROGRAMS:
        _PROGRAMS[key] = _build_program(stage, affine)
    return _PROGRAMS[key]


def _prep_maps(x, word_emb, pos_emb, tok_emb, emb_ln_w, emb_ln_b,
               init_d, init_b, init_M, inter0_d, inter0_b, inter0_M,
               final_d, final_b):
    x = np.asarray(x)
    f = lambda a: np.ascontiguousarray(np.asarray(a), dtype=np.float32)
    h = lambda a: np.ascontiguousarray(a).astype(ml_dtypes.bfloat16)
    affine = bool(np.any(np.asarray(emb_ln_b) != 0))
    w1f = f(init_d) if affine else f(init_d) * f(emb_ln_w)[None, :]
    w1h = w1f.reshape(NC1, 512, KH, 128).transpose(0, 3, 2, 1) \
        .reshape(NC1 * 128, KH * 512)
    w2h = f(inter0_d).reshape(NC1, 512, KI, 128).transpose(0, 3, 2, 1) \
        .reshape(NC1 * 128, KI * 512)
    w3h = f(final_d).reshape(HID, KI, 128).transpose(2, 1, 0) \
        .reshape(128, KI * 512)
    m1h = f(init_M).reshape(NH, SC, 128, S).transpose(0, 2, 1, 3) \
        .reshape(NH * 128, SC * S)
    m2h = f(inter0_M).reshape(NH, SC, 128, S).transpose(0, 2, 1, 3) \
        .reshape(NH * 128, SC * S)
    bcat = np.concatenate([
        f(init_b).reshape(KI, 128).T,
        f(inter0_b).reshape(KI, 128).T,
        f(final_b).reshape(HT, 128).T,
        f(emb_ln_w).reshape(HT, 128).T,
        f(emb_ln_b).reshape(HT, 128).T,
    ], axis=1)
    shared = dict(
        word_emb=f(word_emb),
        posplus=f(pos_emb) + f(tok_emb)[0][None, :],
        bcat=np.ascontiguousarray(bcat),
        w1=h(w1h), w2=h(w2h), w3=h(w3h), m1=h(m1h), m2=h(m2h),
    )
    in_maps = []
    for b in range(B):
        xwb = np.ascontiguousarray(x[b].astype(np.int32).reshape(SC, 128).T)
        in_maps.append(dict(shared, xw=xwb))
    return in_maps, affine


def kernel(**inputs):
    global LAST_RESULT
    stage = os.environ.get("KSTAGE", "full")
    ncores = int(os.environ.get("KCORES", str(N_CORES)))
    in_maps, affine = _prep_maps(**inputs)
    in_maps = in_maps[:ncores]
    nc = _get_program(stage, affine)
    res = run_bass_kernel_spmd(nc, in_maps, list(range(ncores)))
    LAST_RESULT = res
    out = np.stack([res.results[b]["yt"].T for b in range(ncores)])
    if ncores < B:
        out = np.concatenate([out] + [out[:1]] * (B - ncores))
    return out


# revision 46
# speedup vs baseline: 1.0796x; 1.0055x over previous
"""BertTinyFlatten on 8 Trainium2 NeuronCores — data-parallel over batch.

Per core (one batch element):
  emb   = gather(word_emb, x) + (pos_emb + tok_emb[0])      [indirect DMA w/ CCE add]
  x0    = layernorm(emb)                                     [token-major, DVE/ACT]
  x0t   = x0.T (PE transpose, LN affine fused into PSUM->SBUF copy, bf16)
  y1    = x0 @ init_d.T          (token-major out)           [bf16 matmuls]
  y1sq  = (mix(y1, init_M) + b1)^2    -> feature-major       [ACT Square fused]
  y2    = y1sq-chain @ inter0_d.T     -> token-major
  y2sq  = (mix(y2, inter0_M) + b2)^2  -> feature-major
  yt    = final_d-chain @ y2sq + b3   -> feature-major; host transposes back

All matmul operands are bf16 (PSUM accumulate in fp32): the moving
operand streams 1 col/cycle and the 128-col LDWEIGHTS rides the
background weight buffer, giving the 216 ns warm pitch at N=512.
Weights/M-panels are DMA'd as large host-prelaid panels (one trigger
per panel) because each dma_start costs ~0.64 us on the issuing
sequencer; triggers are split between the Sync and Scalar HWDGE
queues and ordered so the chunk-0 gather -> LN -> transpose -> MM
critical path and the w1 panel land first.
"""
import os
import sys

import numpy as np
import ml_dtypes

for _p in ("/opt/trn_rl_repo", "/opt/pypackages"):
    if _p not in sys.path and os.path.isdir(_p):
        sys.path.append(_p)

from contextlib import ExitStack

import concourse.bass as bass
import concourse.tile as tile
from concourse import bacc, masks, mybir
from concourse.bass import IndirectOffsetOnAxis
from concourse.bass_utils import run_bass_kernel_spmd

f32 = mybir.dt.float32
f32r = mybir.dt.float32r
bf16 = mybir.dt.bfloat16
i32 = mybir.dt.int32
AF = mybir.ActivationFunctionType
ALU = mybir.AluOpType
AX = mybir.AxisListType

B, S, HID, NH, INTER, VOCAB = 8, 1024, 512, 8, 2048, 30522
DH = INTER // NH            # 256 features per head
EPS = 1e-12
N_CORES = 8

KH = HID // 128             # 4   k-tiles for dense1
KI = INTER // 128           # 16  k-tiles for dense2/3
SC = S // 128               # 8   token chunks
NC1 = INTER // 512          # 4   n-chunks (512) for dense1/2
HT = HID // 128             # 4   hid tiles for dense3

STAGES = ("A", "B", "C", "D", "E", "full")


def _build_program(stage="full", affine=False):
    upto = STAGES.index(stage)
    nc = bacc.Bacc("TRN2", target_bir_lowering=False, debug=False,
                   num_devices=N_CORES, num_swdge_queues=2)

    xw = nc.dram_tensor("xw", [128, SC], i32, kind="ExternalInput").ap()
    word_emb = nc.dram_tensor("word_emb", [VOCAB, HID], bf16, kind="ExternalInput").ap()
    posplus = nc.dram_tensor("posplus", [S, HID], bf16, kind="ExternalInput").ap()
    # biases+ln catted: [0:16]=b1, [16:32]=b2, [32:36]=b3, [36:40]=lnw, [40:44]=lnb
    bcat = nc.dram_tensor("bcat", [128, 2 * KI + HT + 2 * HT], f32,
                          kind="ExternalInput").ap()
    # prelaid panels (host does the tiling): see _prep_maps for layouts
    w1 = nc.dram_tensor("w1", [NC1 * 128, KH * 512], bf16, kind="ExternalInput").ap()
    m1 = nc.dram_tensor("m1", [NH * 128, SC * S], bf16, kind="ExternalInput").ap()
    w2 = nc.dram_tensor("w2", [NC1 * 128, KI * 512], bf16, kind="ExternalInput").ap()
    m2 = nc.dram_tensor("m2", [NH * 128, SC * S], bf16, kind="ExternalInput").ap()
    w3 = nc.dram_tensor("w3", [128, KI * 512], bf16, kind="ExternalInput").ap()
    yt_out = nc.dram_tensor("yt", [HID, S], bf16, kind="ExternalOutput").ap()

    with tile.TileContext(nc) as tc, ExitStack() as ctx:
        pool = ctx.enter_context(tc.tile_pool(name="sbuf", bufs=1))
        psum = ctx.enter_context(tc.tile_pool(name="psum", bufs=1, space="PSUM"))

        # ---- startup: critical path first -------------------------------
        t_idx = pool.tile([128, SC], i32)
        nc.sync.dma_start(t_idx[:], xw[:])

        # biases early: tiny payload, must not queue behind panel traffic
        t_bc = pool.tile([128, 2 * KI + 3 * HT], f32)
        nc.sync.dma_start(t_bc[:], bcat[:])

        emb = [pool.tile([128, HID], bf16, tag="emb", bufs=SC, name=f"emb{c}")
               for c in range(SC)]
        nc.sync.dma_start(emb[0][:], posplus[0:128, :])

        # w1 panel next (quarter n=0 first); pp chunks 1-7 after it — their
        # gathers are serialized on the gpsimd descriptor builds anyway
        w1sb = pool.tile([128, NC1 * KH * 512], bf16)
        for n in range(NC1):
            nc.sync.dma_start(w1sb[:, n * 2048:(n + 1) * 2048],
                              w1[n * 128:(n + 1) * 128, :])
        for c in range(1, SC):
            nc.sync.dma_start(emb[c][:], posplus[c * 128:(c + 1) * 128, :])

        # gathers (gpsimd swdge): chunk 0 split across both queues for latency
        def gather(out_ap, idx_ap, queue=None):
            gi = nc.gpsimd.indirect_dma_start(
                out=out_ap, out_offset=None,
                in_=word_emb[:out_ap.partition_size(), :],
                in_offset=IndirectOffsetOnAxis(ap=idx_ap, axis=0),
                bounds_check=VOCAB - 1, oob_is_err=False,
                compute_op=ALU.add,
            )
            if queue:
                gi.ins.queue = queue
            return gi

        # chunk 0: gather into its own tile (no CCE add) so it can run in
        # parallel with the posplus DMA; summed on DVE below.
        g0 = pool.tile([128, HID], bf16)
        gather0 = nc.gpsimd.indirect_dma_start(
            out=g0[:], out_offset=None, in_=word_emb[:128, :],
            in_offset=IndirectOffsetOnAxis(ap=t_idx[:, 0:1], axis=0),
            bounds_check=VOCAB - 1, oob_is_err=False)
        for c in range(1, SC):
            gather(emb[c][:], t_idx[:, c:c + 1],
                   queue="qPoolDynamic1" if c % 2 else None)

        # ---- constants --------------------------------------------------
        ident = pool.tile([128, 128], bf16)
        masks.make_identity(nc, ident[:])
        zerocol = pool.tile([128, 1], f32)
        nc.vector.memset(zerocol[:], 0.0)
        epscol = pool.tile([128, 1], f32)
        nc.vector.memset(epscol[:], EPS)
        # column bases into t_bc: b1, b2, b3, lnw, lnb
        B1, B2, B3, LNW, LNB = 0, KI, 2 * KI, 2 * KI + HT, 2 * KI + 2 * HT

        def bc(base, i):
            return t_bc[:, base + i:base + i + 1]

        # dummy Sqrt first so walrus picks the sqrt table set once (it also
        # contains Square/Identity) instead of reloading mid-layernorm
        actwarm = pool.tile([128, 1], f32)
        nc.scalar.activation(actwarm[:], epscol[:], AF.Sqrt, bias=zerocol[:])

        # PE warmup: ramp the clock right before dense1 (g0-gated ones land
        # in the ~12-16us window, back-to-back with the first transposes)
        warmps = psum.tile([128, 512], f32, tag="mm", bufs=8, name="warm")
        for i in range(4):
            nc.tensor.transpose(warmps[:, 0:64].bitcast(bf16), ident[:], ident[:])
        for i in range(16):
            nc.tensor.transpose(warmps[:, 0:64].bitcast(bf16), g0[:, 0:128],
                                ident[:])

        # m panels: DMA enqueue gated behind gather progress (via tiny gpsimd
        # copies into the slot) so 2MB panels don't delay gather payloads on
        # the shared DMA rings.
        def m_panel(m_ap, h, nm, gate=None):
            t = pool.tile([128, SC * S], bf16, tag="m", bufs=3, name=f"{nm}p{h}")
            if gate is not None:
                nc.gpsimd.tensor_copy(t[:, 0:1], gate[:, 0:1])
            nc.sync.dma_start(t[:], m_ap[h * 128:(h + 1) * 128, :])
            return t

        m1pan = [m_panel(m1, 0, "m1", gate=emb[7]),
                 m_panel(m1, 1, "m1", gate=emb[7]),
                 m_panel(m1, 2, "m1", gate=emb[7])]

        # ---- stage A: per-chunk layernorm (invoked from fused dense1) --
        def layernorm_chunk(c):
            e = emb[c]
            msum = pool.tile([128, 1], f32, tag="msum", bufs=2, name=f"msum{c}")
            if c == 0:
                # fused: e = e + g0 (gather part), msum = sum(e)
                nc.vector.scalar_tensor_tensor(e[:], e[:], 1.0, g0[:],
                                               op0=ALU.mult, op1=ALU.add,
                                               accum_out=msum[:])
            else:
                nc.vector.reduce_sum(msum[:], e[:], axis=AX.X)
            sqd = pool.tile([128, HID], bf16, tag="sqd", bufs=2, name=f"sqd{c}")
            ssq = pool.tile([128, 1], f32, tag="ssq", bufs=2, name=f"ssq{c}")
            nc.scalar.activation(sqd[:], e[:], AF.Square, bias=zerocol[:],
                                 accum_out=ssq[:])
            # var = ssq/H - (msum/H)^2 ; rstd = rsqrt(var+eps)
            t1 = pool.tile([128, 1], f32, tag="t1", bufs=2, name=f"t1_{c}")
            nc.vector.tensor_scalar(t1[:], msum[:], msum[:], 1.0 / (HID * HID),
                                    op0=ALU.mult, op1=ALU.mult)
            var = pool.tile([128, 1], f32, tag="var", bufs=2, name=f"var{c}")
            nc.vector.tensor_scalar(var[:], ssq[:], 1.0 / HID, t1[:],
                                    op0=ALU.mult, op1=ALU.subtract)
            std = pool.tile([128, 1], f32, tag="std", bufs=2, name=f"std{c}")
            nc.scalar.activation(std[:], var[:], AF.Sqrt, bias=epscol[:])
            rstd = pool.tile([128, 1], f32, tag="rstd", bufs=2, name=f"rstd{c}")
            nc.vector.reciprocal(rstd[:], std[:])
            # e = e*rstd - (msum/H)*rstd
            mr = pool.tile([128, 1], f32, tag="mr", bufs=2, name=f"mr{c}")
            nc.vector.tensor_scalar(mr[:], msum[:], rstd[:], 1.0 / HID,
                                    op0=ALU.mult, op1=ALU.mult)
            nc.vector.tensor_scalar(e[:], e[:], rstd[:], mr[:],
                                    op0=ALU.mult, op1=ALU.subtract)

        # feature-major activations live in the 16-slot "featmaj" ring:
        # x0t (4 tiles) -> y1sq (16) -> y2sq (16), WAR-serialized by Tile.
        x0t = []
        for ht in range(HT):
            x0t.append(pool.tile([128, S], bf16, tag="featmaj", bufs=16,
                                 name=f"x0t{ht}"))

        def mix(yin, m_ap, panels, bias_base, out_name, tail_hooks=None):
            # per-head seq mix + bias + square; token-major in, feature-major out
            ysq = []
            for h in range(NH):
                pan = panels[h]
                groups = [[None] * 2 for _ in range(2)]
                for tc_i in range(2):
                    for dp in range(2):
                        groups[tc_i][dp] = psum.tile(
                            [128, 512], f32, tag="mm", bufs=8,
                            name=f"{out_name}p{h}_{tc_i}_{dp}")
                for s in range(SC):
                    for dp in range(2):
                        lhsT = yin[s][:, h * DH + dp * 128: h * DH + (dp + 1) * 128]
                        for tc_i in range(2):
                            nc.tensor.matmul(
                                groups[tc_i][dp][:], lhsT,
                                pan[:, s * S + tc_i * 512: s * S + (tc_i + 1) * 512],
                                start=(s == 0), stop=(s == SC - 1))
                if h + 3 < NH:
                    panels.append(m_panel(m_ap, h + 3, out_name))
                if tail_hooks and h in tail_hooks:
                    for fn in tail_hooks[h]:
                        fn()
                for dp in range(2):
                    i = h * 2 + dp
                    yo = pool.tile([128, S], bf16, tag="featmaj", bufs=16,
                                   name=f"{out_name}{i}")
                    for tc_i in range(2):
                        nc.scalar.activation(yo[:, tc_i * 512:(tc_i + 1) * 512],
                                             groups[tc_i][dp][:], AF.Square,
                                             bias=bc(bias_base, i))
                    ysq.append(yo)
            return ysq

        def w2_panel(n):
            t = pool.tile([128, KI * 512], bf16, tag="w2", bufs=2, name=f"w2q{n}")
            nc.sync.dma_start(t[:], w2[n * 128:(n + 1) * 128, :])
            return t

        def dense1(xin, transpose_src):
            # token-major out: y[s, n] = x @ w1; s-major with fused LN+transpose
            yt = []
            for s in range(SC):
                yt.append(pool.tile([128, INTER], bf16, tag="tokmaj", bufs=SC,
                                    name=f"y1_{s}"))
            for s in range(SC):
                layernorm_chunk(s)
                for ht in range(HT):
                    pt = psum.tile([128, 512], f32, tag="mm", bufs=8,
                                   name=f"ptr{ht}_{s}")
                    ptb = pt[:, 0:64].bitcast(bf16)
                    nc.tensor.transpose(
                        ptb,
                        transpose_src[s][:, ht * 128:(ht + 1) * 128],
                        ident[:])
                    if affine:
                        nc.vector.tensor_scalar(
                            xin[ht][:, s * 128:(s + 1) * 128], ptb,
                            bc(LNW, ht), bc(LNB, ht),
                            op0=ALU.mult, op1=ALU.add)
                    else:
                        # ln_w folded into w1 host-side; ln_b known zero
                        nc.vector.tensor_copy(
                            xin[ht][:, s * 128:(s + 1) * 128], ptb)
                for n in range(NC1):
                    ps = psum.tile([128, 512], f32, tag="mm", bufs=8,
                                   name=f"y1p{n}_{s}")
                    for k in range(KH):
                        nc.tensor.matmul(
                            ps[:], xin[k][:, s * 128:(s + 1) * 128],
                            w1sb[:, n * 2048 + k * 512: n * 2048 + (k + 1) * 512],
                            start=(k == 0), stop=(k == KH - 1))
                    nc.scalar.copy(yt[s][:, n * 512:(n + 1) * 512], ps[:])
            return yt

        def dense2(xin, w2pans, tail_hooks=None):
            yt = []
            for s in range(SC):
                yt.append(pool.tile([128, INTER], bf16, tag="tokmaj", bufs=SC,
                                    name=f"y2_{s}"))
            for n in range(NC1):
                wq = w2pans[n]
                for s in range(SC):
                    ps = psum.tile([128, 512], f32, tag="mm", bufs=8,
                                   name=f"y2p{n}_{s}")
                    for k in range(KI):
                        nc.tensor.matmul(
                            ps[:], xin[k][:, s * 128:(s + 1) * 128],
                            wq[:, k * 512:(k + 1) * 512],
                            start=(k == 0), stop=(k == KI - 1))
                    nc.scalar.copy(yt[s][:, n * 512:(n + 1) * 512], ps[:])
                if n + 2 < NC1:
                    w2pans.append(w2_panel(n + 2))
                if tail_hooks and n in tail_hooks:
                    for fn in tail_hooks[n]:
                        fn()
            return yt

        def dump(tiles, width=S):
            # debug: write four [128, >=width] tiles to yt_out (via staging)
            for i, t in enumerate(tiles[:4]):
                yo = pool.tile([128, S], bf16, tag="out", bufs=2, name=f"dmp{i}")
                nc.vector.tensor_copy(yo[:, 0:width], t[:, 0:width])
                nc.sync.dma_start(yt_out[i * 128:(i + 1) * 128, 0:width],
                                  yo[:, 0:width])

        if upto == 0:                       # stage A only
            y1 = dense1(x0t, emb)
            dump(x0t)
        if upto >= 1:
            y1 = dense1(x0t, emb)
            if upto == 1:
                dump(y1)
        if upto >= 2:
            mix2_panels = []
            w2pans = []
            hooks1 = {
                5: [lambda: w2pans.append(w2_panel(0))],
                6: [lambda: w2pans.append(w2_panel(1))],
            }
            y1sq = mix(y1, m1, m1pan, B1, "y1sq", tail_hooks=hooks1)
            if upto == 2:
                dump(y1sq)
        if upto >= 3:
            w3sb = pool.tile([128, KI * 512], bf16)
            hooks2 = {
                0: [lambda: mix2_panels.append(m_panel(m2, 0, "m2"))],
                1: [lambda: mix2_panels.append(m_panel(m2, 1, "m2"))],
                2: [lambda: nc.sync.dma_start(w3sb[:], w3[:]),
                    lambda: mix2_panels.append(m_panel(m2, 2, "m2"))],
            }
            y2 = dense2(y1sq, w2pans, tail_hooks=hooks2)
            if upto == 3:
                dump(y2)
        if upto >= 4:
            y2sq = mix(y2, m2, mix2_panels, B2, "y2sq")
            if upto == 4:
                dump(y2sq)
        if upto >= 5:                       # dense3 + bias + store
            for ht in range(HT):
                yo = pool.tile([128, S], bf16, tag="out", bufs=2, name=f"yt{ht}")
                pss = [psum.tile([128, 512], f32, tag="mm", bufs=8,
                                 name=f"p3_{ht}_{sc}") for sc in range(2)]
                for k in range(KI):
                    for sc in range(2):
                        nc.tensor.matmul(
                            pss[sc][:],
                            w3sb[:, k * 512 + ht * 128: k * 512 + ht * 128 + 128],
                            y2sq[k][:, sc * 512:(sc + 1) * 512],
                            start=(k == 0), stop=(k == KI - 1))
                for sc in range(2):
                    nc.scalar.activation(yo[:, sc * 512:(sc + 1) * 512], pss[sc][:],
                                         AF.Identity, bias=bc(B3, ht))
                    nc.sync.dma_start(
                        yt_out[ht * 128:(ht + 1) * 128, sc * 512:(sc + 1) * 512],
                        yo[:, sc * 512:(sc + 1) * 512])

    nc.compile()
    return nc


_PROGRAMS = {}
LAST_RESULT = None


def _get_program(stage="full", affine=False):
    key = (stage, affine)
    if key not in _PROGRAMS:
        _PROGRAMS[key] = _build_program(stage, affine)
    return _PROGRAMS[key]


def _prep_maps(x, word_emb, pos_emb, tok_emb, emb_ln_w, emb_ln_b,
               init_d, init_b, init_M, inter0_d, inter0_b, inter0_M,
               final_d, final_b):
    x = np.asarray(x)
    f = lambda a: np.ascontiguousarray(np.asarray(a), dtype=np.float32)
    h = lambda a: np.ascontiguousarray(a).astype(ml_dtypes.bfloat16)
    affine = bool(np.any(np.asarray(emb_ln_b) != 0))
    w1f = f(init_d) if affine else f(init_d) * f(emb_ln_w)[None, :]
    w1h = w1f.reshape(NC1, 512, KH, 128).transpose(0, 3, 2, 1) \
        .reshape(NC1 * 128, KH * 512)
    w2h = f(inter0_d).reshape(NC1, 512, KI, 128).transpose(0, 3, 2, 1) \
        .reshape(NC1 * 128, KI * 512)
    w3h = f(final_d).reshape(HID, KI, 128).transpose(2, 1, 0) \
        .reshape(128, KI * 512)
    m1h = f(init_M).reshape(NH, SC, 128, S).transpose(0, 2, 1, 3) \
        .reshape(NH * 128, SC * S)
    m2h = f(inter0_M).reshape(NH, SC, 128, S).transpose(0, 2, 1, 3) \
        .reshape(NH * 128, SC * S)
    bcat = np.concatenate([
        f(init_b).reshape(KI, 128).T,
        f(inter0_b).reshape(KI, 128).T,
        f(final_b).reshape(HT, 128).T,
        f(emb_ln_w).reshape(HT, 128).T,
        f(emb_ln_b).reshape(HT, 128).T,
    ], axis=1)
    shared = dict(
        word_emb=h(f(word_emb)),
        posplus=h(f(pos_emb) + f(tok_emb)[0][None, :]),
        bcat=np.ascontiguousarray(bcat),
        w1=h(w1h), w2=h(w2h), w3=h(w3h), m1=h(m1h), m2=h(m2h),
    )
    in_maps = []
    for b in range(B):
        xwb = np.ascontiguousarray(x[b].astype(np.int32).reshape(SC, 128).T)
        in_maps.append(dict(shared, xw=xwb))
    return in_maps, affine


def kernel(**inputs):
    global LAST_RESULT
    stage = os.environ.get("KSTAGE", "full")
    ncores = int(os.environ.get("KCORES", str(N_CORES)))
    in_maps, affine = _prep_maps(**inputs)
    in_maps = in_maps[:ncores]
    nc = _get_program(stage, affine)
    res = run_bass_kernel_spmd(nc, in_maps, list(range(ncores)))
    LAST_RESULT = res
    out = np.stack([np.asarray(res.results[b]["yt"], dtype=np.float32).T
                    for b in range(ncores)])
    if ncores < B:
        out = np.concatenate([out] + [out[:1]] * (B - ncores))
    return out
